# revision 8
# baseline (speedup 1.0000x reference)
"""Trainium2 Bass kernel for a 2-layer GATv2 encoder (gnn_message_passing).

Self-contained: hardcodes problem shapes from the task spec.

Sharding: nodes split into 8 contiguous ranges (6250/core); edges assigned to
the core owning their dst, sorted by dst. Each core runs an identical SPMD
program over a uniform block structure (50 blocks x 125 dst slots x 2304
padded edge slots). Node features are projected shard-locally and the
src-side projection table is AllGathered so每 core can gather arbitrary rows.

Device algorithm (per layer, |att|-scaled feature space):
  m_T[f,e] = we_sc^T eattr + xr_sc_blk^T A_T + xl_sc[src]^T      (PSUM accum)
  score    = sgn^T relu(m) - 0.2 sgn^T relu(-m)                  (lrelu identity)
  w        = exp(score)            (no max subtraction; scores are O(10))
  num,den  = A^T [w*xl | w | eattr,1]                            (one-hot matmul)
  out      = mean_h (num_h / den_h / |att_h|) + bo  (late division, self-loops
             added densely per node with device-computed mean edge_attr)
"""
import sys
for _p in ('/opt/trn_rl_repo', '/root/.axon_site/_ro/trn_rl_repo'):
    if _p not in sys.path:
        sys.path.insert(0, _p)

from dataclasses import dataclass, field
import numpy as np

import concourse.bass as bass
import concourse.bacc as bacc
import concourse.mybir as mybir
import concourse.tile as tile

F32 = mybir.dt.float32
I32 = mybir.dt.int32
AF = mybir.ActivationFunctionType
OP = mybir.AluOpType
NEG = 0.2


@dataclass
class Cfg:
    N: int = 50000
    E: int = 800000
    IN: int = 512
    H: int = 2
    C: int = 64
    ED: int = 3
    NCORES: int = 8
    BLKN: int = 125                      # dst slots per block
    NBLK: int = 50                       # blocks per core
    chunks: tuple = (512, 512, 512, 512, 256)   # edge chunk sizes per block

    @property
    def NSH(self):
        return self.BLKN * self.NBLK

    @property
    def HC(self):
        return self.H * self.C

    @property
    def EBLK(self):
        return sum(self.chunks)

    @property
    def NS(self):
        return self.NBLK * self.EBLK     # edge slots per core

FULL = Cfg()


# --------------------------------------------------------------------------
# Host-side staging (index preprocessing + weight scaling). Pure numpy.
# --------------------------------------------------------------------------
def stage_inputs(cfg, inputs):
    N, E, IN, H, C, ED = cfg.N, cfg.E, cfg.IN, cfg.H, cfg.C, cfg.ED
    HC, NSH, BLKN, NBLK, EBLK, NS = cfg.HC, cfg.NSH, cfg.BLKN, cfg.NBLK, cfg.EBLK, cfg.NS
    NC = cfg.NCORES

    x = np.asarray(inputs['x'], np.float32)
    src = np.asarray(inputs['edge_index'][0], np.int64)
    dst = np.asarray(inputs['edge_index'][1], np.int64)
    eattr = np.asarray(inputs['edge_attr'], np.float32)

    order = np.argsort(dst, kind='stable')
    src_s, dst_s, ea_s = src[order], dst[order], eattr[order]
    core_bounds = np.searchsorted(dst_s, np.arange(NC + 1) * NSH)

    def scale_params(wl, bl, wr, br, we, att):
        a = np.asarray(att, np.float32).reshape(HC)
        s = np.abs(a)
        sgn = np.zeros((HC, H), np.float32)
        for h in range(H):
            sgn[h * C:(h + 1) * C, h] = np.sign(a[h * C:(h + 1) * C])
        return (np.asarray(wl, np.float32) * s, np.asarray(bl, np.float32) * s,
                np.asarray(wr, np.float32) * s, np.asarray(br, np.float32) * s,
                np.asarray(we, np.float32) * s, sgn, s)

    P = {k: np.asarray(v, np.float32) for k, v in inputs.items()
         if k not in ('x', 'edge_index', 'edge_attr')}
    wl1s, bl1s, wr1s, br1s, we1s, sgn1, s1 = scale_params(P['wl1'], P['bl1'], P['wr1'], P['br1'], P['we1'], P['att1'])
    wl2s, bl2s, wr2s, br2s, we2s, sgn2, s2 = scale_params(P['wl2'], P['bl2'], P['wr2'], P['br2'], P['we2'], P['att2'])

    def rep(v, rows=128):                      # replicate a row vector
        return np.tile(np.asarray(v, np.float32)[None, :], (rows, 1))

    def warr(w):                               # [IN,HC] -> [128, (IN//128)*HC]
        icn = w.shape[0] // 128
        return np.ascontiguousarray(
            w.reshape(icn, 128, HC).transpose(1, 0, 2).reshape(128, icn * HC))

    consts = {
        'wlr1': warr(wl1s), 'wrr1': warr(wr1s), 'we1': we1s.copy(),
        'wl2': wl2s.copy(), 'wr2': wr2s.copy(), 'we2': we2s.copy(),
        'bl1b': rep(bl1s), 'br1b': rep(br1s), 'bl2b': rep(bl2s), 'br2b': rep(br2s),
        'ia1b': rep(0.5 / np.maximum(s1, 1e-20)), 'ia2b': rep(0.5 / np.maximum(s2, 1e-20)),
        'bo1b': rep(P['bo1']), 'bo2b': rep(P['bo2']), 'prelub': rep(P['prelu_w']),
        'sgn1': sgn1, 'sgn1n': (-NEG) * sgn1, 'sgn2': sgn2, 'sgn2n': (-NEG) * sgn2,
        'iotac': np.arange(128, dtype=np.float32).reshape(128, 1),
        'iotar': np.tile(np.arange(128, dtype=np.float32)[None, :], (128, 1)),
        'ident': np.eye(128, dtype=np.float32),
    }

    in_maps = []
    for c in range(NC):
        e0, e1 = core_bounds[c], core_bounds[c + 1]
        srcC = src_s[e0:e1]
        dstC = dst_s[e0:e1] - c * NSH
        eaC = ea_s[e0:e1]
        slots_src = np.zeros(NS, np.int32)
        slots_dst = np.full(NS, -1.0, np.float32)
        slots_ea = np.zeros((NS, ED), np.float32)
        slots_one = np.zeros(NS, np.float32)
        blk_of = dstC // BLKN
        bb = np.searchsorted(blk_of, np.arange(NBLK + 1))
        for b in range(NBLK):
            m0, m1 = bb[b], bb[b + 1]
            ne = m1 - m0
            assert ne <= EBLK, f'core {c} block {b}: {ne} > {EBLK}'
            o = b * EBLK
            slots_src[o:o + ne] = srcC[m0:m1]
            slots_dst[o:o + ne] = (dstC[m0:m1] - b * BLKN).astype(np.float32)
            slots_ea[o:o + ne] = eaC[m0:m1]
            slots_one[o:o + ne] = 1.0
        eaug = np.concatenate([slots_ea, slots_one[:, None]], axis=1)     # [NS,4]
        m = {
            'xT': np.ascontiguousarray(x[c * NSH:(c + 1) * NSH].T) if IN == x.shape[1] else None,
            'srci': np.ascontiguousarray(slots_src.reshape(-1, 128).T),   # [128, NS//128]
            'dcol': np.ascontiguousarray(slots_dst.reshape(-1, 128).T),   # [128, NS//128]
            'drow': slots_dst.reshape(1, NS).copy(),
            'eaT': np.ascontiguousarray(slots_ea.T),                      # [ED, NS]
            'eaug': np.ascontiguousarray(
                eaug.reshape(-1, 128, ED + 1).transpose(1, 0, 2).reshape(128, -1)),
            **consts,
        }
        in_maps.append(m)
    return in_maps


# --------------------------------------------------------------------------
# Device program (identical for all cores).
# --------------------------------------------------------------------------
def build_program(cfg):
    N, IN, H, C, ED = cfg.N, cfg.IN, cfg.H, cfg.C, cfg.ED
    HC, NSH, BLKN, NBLK, EBLK, NS = cfg.HC, cfg.NSH, cfg.BLKN, cfg.NBLK, cfg.EBLK, cfg.NS
    NC = cfg.NCORES
    ICN = IN // 128                       # contraction chunks for layer-1 projection
    NT = EBLK // 128                      # edge tiles per block

    nc = bacc.Bacc('TRN2', target_bir_lowering=False, debug=False, num_devices=NC)

    D = {}
    def din(name, shape, dt=F32):
        D[name] = nc.dram_tensor(name, list(shape), dt, kind='ExternalInput')
        return D[name]

    din('xT', [IN, NSH]); din('srci', [128, NS // 128], I32)
    din('dcol', [128, NS // 128]); din('drow', [1, NS]); din('eaT', [ED, NS])
    din('eaug', [128, (NS // 128) * (ED + 1)])
    din('wlr1', [128, ICN * HC]); din('wrr1', [128, ICN * HC]); din('we1', [ED, HC])
    din('wl2', [C, HC]); din('wr2', [C, HC]); din('we2', [ED, HC])
    for nm in ('bl1b', 'br1b', 'bl2b', 'br2b', 'ia1b', 'ia2b'):
        din(nm, [128, HC])
    for nm in ('bo1b', 'bo2b', 'prelub'):
        din(nm, [128, C])
    for nm in ('sgn1', 'sgn1n', 'sgn2', 'sgn2n'):
        din(nm, [128, H])
    din('iotac', [128, 1]); din('iotar', [128, 128]); din('ident', [128, 128])
    y_d = nc.dram_tensor('y', [NSH, C], F32, kind='ExternalOutput')

    with tile.TileContext(nc) as tc:
        cp = tc.alloc_tile_pool(name='consts', bufs=1)
        rp = tc.alloc_tile_pool(name='resident', bufs=1)
        dp = tc.alloc_tile_pool(name='dram', bufs=1, space='DRAM')

        CB = {}
        for nm, t in D.items():
            if nm in ('xT', 'srci', 'dcol', 'drow', 'eaT', 'eaug'):
                continue
            sh = list(t.shape)
            CB[nm] = cp.tile(sh, t.dtype, tag=nm, name=f'cb_{nm}')
            nc.sync.dma_start(out=CB[nm][:], in_=t[:])

        xr_shared = rp.tile([BLKN, NBLK * HC], F32, tag='xr', name='xr_res')
        xr_res = {1: xr_shared, 2: xr_shared}
        h_res = rp.tile([BLKN, NBLK * C], F32, tag='h_res', name='h_res')
        mean_r = rp.tile([BLKN, NBLK * ED], F32, tag='mean_r', name='mean_r')

        xl_bounce = {1: dp.tile([NSH, HC], F32, tag='xlb1', name='xlb1'),
                     2: dp.tile([NSH, HC], F32, tag='xlb2', name='xlb2')}
        xl_table = {1: dp.tile([N, HC], F32, addr_space='Shared', tag='xlt1', name='xlt1'),
                    2: dp.tile([N, HC], F32, addr_space='Shared', tag='xlt2', name='xlt2')}

        # ---------------- layer-1 projections + AllGather ----------------
        with (tc.tile_pool(name='pj', bufs=3) as pj,
              tc.tile_pool(name='pjp', bufs=2, space='PSUM') as pjp):
            for b in range(NBLK):
                ps_xl = pjp.tile([BLKN, HC], F32, space='PSUM', tag='ps_xl')
                ps_xr = pjp.tile([BLKN, HC], F32, space='PSUM', tag='ps_xr')
                for ic in range(ICN):
                    xt = pj.tile([128, BLKN], F32, tag='xt')
                    nc.sync.dma_start(out=xt[:], in_=D['xT'][128 * ic:128 * (ic + 1),
                                                             BLKN * b:BLKN * (b + 1)])
                    nc.tensor.matmul(ps_xl[:], lhsT=xt[:], rhs=CB['wlr1'][:, HC * ic:HC * (ic + 1)],
                                     start=(ic == 0), stop=(ic == ICN - 1))
                    nc.tensor.matmul(ps_xr[:], lhsT=xt[:], rhs=CB['wrr1'][:, HC * ic:HC * (ic + 1)],
                                     start=(ic == 0), stop=(ic == ICN - 1))
                xl_sb = pj.tile([BLKN, HC], F32, tag='xl_sb')
                nc.vector.tensor_tensor(out=xl_sb[:], in0=ps_xl[:], in1=CB['bl1b'][:BLKN, :],
                                        op=OP.add)
                nc.sync.dma_start(out=xl_bounce[1][BLKN * b:BLKN * (b + 1), :], in_=xl_sb[:])
                nc.vector.tensor_tensor(out=xr_res[1][:, HC * b:HC * (b + 1)], in0=ps_xr[:],
                                        in1=CB['br1b'][:BLKN, :], op=OP.add)

        nc.gpsimd.collective_compute(
            'AllGather', OP.bypass, replica_groups=[list(range(NC))],
            ins=[xl_bounce[1][:]], outs=[xl_table[1][:]])

        # ---------------- edge pass (shared for both layers) ----------------
        def edge_pass(layer):
            we_sb = CB['we1' if layer == 1 else 'we2']
            sgn_sb = CB['sgn1' if layer == 1 else 'sgn2']
            sgnn_sb = CB['sgn1n' if layer == 1 else 'sgn2n']
            ia_sb = CB['ia1b' if layer == 1 else 'ia2b']
            bo_sb = CB['bo1b' if layer == 1 else 'bo2b']
            xr = xr_res[layer]
            tbl = xl_table[layer]
            SEC = HC + H + (ED + 1 if layer == 1 else 0)     # 134 / 130
            base_row = None  # filled by in_maps; table rows for own shard are c*NSH.. but
            # we address own rows via partition id? No: own shard rows in the AllGather
            # output are at offset rank*NSH. We cannot know the rank in an SPMD program
            # without partition_id, so instead re-read own xl from the local bounce buffer.
            xl_own_src = xl_bounce[layer]

            with (tc.tile_pool(name=f'ep{layer}', bufs=2) as sb,
                  tc.tile_pool(name=f'epb{layer}', bufs=2) as sbb,
                  tc.tile_pool(name=f'epb1{layer}', bufs=1) as sb1,
                  tc.tile_pool(name=f'eps{layer}', bufs=3) as sbc,
                  tc.tile_pool(name=f'epp{layer}', bufs=2, space='PSUM') as pp,
                  tc.tile_pool(name=f'epps{layer}', bufs=2, space='PSUM') as spp,
                  tc.tile_pool(name=f'eppm{layer}', bufs=2, space='PSUM') as ppm):
                for b in range(NBLK):
                    t0 = b * (NS // 128 // NBLK)             # first tile col of block
                    ncols = NT
                    srci_sb = sbb.tile([128, NT], I32, tag='srci')
                    nc.sync.dma_start(out=srci_sb[:], in_=D['srci'][:, t0:t0 + ncols])
                    dcol_sb = sbb.tile([128, NT], F32, tag='dcol')
                    nc.sync.dma_start(out=dcol_sb[:], in_=D['dcol'][:, t0:t0 + ncols])
                    drow_sb = sb1.tile([1, EBLK], F32, tag='drow')
                    nc.sync.dma_start(out=drow_sb[:], in_=D['drow'][:, b * EBLK:(b + 1) * EBLK])
                    eaT_sb = sb1.tile([ED, EBLK], F32, tag='eaT')
                    nc.sync.dma_start(out=eaT_sb[:], in_=D['eaT'][:, b * EBLK:(b + 1) * EBLK])

                    drow_b = sb1.tile([128, EBLK], F32, tag='drow_b')
                    nc.gpsimd.partition_broadcast(drow_b[:], drow_sb[:])
                    at_sb = sb1.tile([128, EBLK], F32, tag='at')
                    nc.vector.tensor_scalar(out=at_sb[:], in0=drow_b[:], scalar1=CB['iotac'][:],
                                            scalar2=None, op0=OP.is_equal)

                    xlg = sbb.tile([128, EBLK], F32, tag='xlg')       # [e, f] 18 tiles
                    a_sb = sbb.tile([128, EBLK], F32, tag='a')        # [e, j] 18 tiles
                    stg = sbb.tile([128, NT * SEC], F32, tag='stg')
                    if layer == 1:
                        ev = stg[:].rearrange('p (k c) -> p k c', k=NT)[:, :, HC + H:SEC]
                        nc.sync.dma_start(
                            out=ev, in_=D['eaug'][:, t0 * (ED + 1):(t0 + ncols) * (ED + 1)])

                    nd = pp.tile([128, SEC], F32, space='PSUM', tag='nd')
                    score = pp.tile([128, 2 * NT], F32, space='PSUM', tag='score')

                    for k in range(NT):
                        nc.gpsimd.indirect_dma_start(
                            out=xlg[:, 128 * k:128 * (k + 1)], out_offset=None,
                            in_=tbl[:],
                            in_offset=bass.IndirectOffsetOnAxis(ap=srci_sb[:, k:k + 1], axis=0))
                        nc.vector.tensor_scalar(out=a_sb[:, 128 * k:128 * (k + 1)],
                                                in0=CB['iotar'][:], scalar1=dcol_sb[:, k:k + 1],
                                                scalar2=None, op0=OP.is_equal)

                    off = 0
                    for ci, csz in enumerate(cfg.chunks):
                        ct = csz // 128
                        k0 = off // 128
                        m_ps = ppm.tile([128, 512], F32, space='PSUM', tag='m')
                        nc.tensor.matmul(m_ps[:, :csz], lhsT=we_sb[:], rhs=eaT_sb[:, off:off + csz],
                                         start=True, stop=False)
                        nc.tensor.matmul(m_ps[:, :csz], lhsT=xr[:, HC * b:HC * (b + 1)],
                                         rhs=at_sb[:BLKN, off:off + csz], start=False, stop=False)
                        for k in range(ct):
                            nc.tensor.matmul(m_ps[:, 128 * k:128 * (k + 1)],
                                             lhsT=xlg[:, 128 * (k0 + k):128 * (k0 + k + 1)],
                                             rhs=CB['ident'][:], start=False,
                                             stop=(k == ct - 1), is_transpose=True)
                        rpos = sbc.tile([128, 512], F32, tag='rpos')
                        rneg = sbc.tile([128, 512], F32, tag='rneg')
                        nc.scalar.activation(out=rpos[:, :csz], in_=m_ps[:, :csz], func=AF.Relu)
                        nc.scalar.activation(out=rneg[:, :csz], in_=m_ps[:, :csz], func=AF.Relu,
                                             scale=-1.0)
                        for k in range(ct):
                            sc = score[:, 2 * (k0 + k):2 * (k0 + k) + 2]
                            nc.tensor.matmul(sc, lhsT=rpos[:, 128 * k:128 * (k + 1)],
                                             rhs=sgn_sb[:], start=True, stop=False)
                            nc.tensor.matmul(sc, lhsT=rneg[:, 128 * k:128 * (k + 1)],
                                             rhs=sgnn_sb[:], start=False, stop=True)
                        off += csz

                    wv = stg[:].rearrange('p (k c) -> p k c', k=NT)[:, :, HC:HC + H]
                    nc.scalar.activation(out=wv, in_=score[:], func=AF.Exp)
                    xv = xlg[:].rearrange('p (k c) -> p k c', k=NT)
                    sv = stg[:].rearrange('p (k c) -> p k c', k=NT)
                    for h in range(H):
                        nc.vector.tensor_tensor(
                            out=sv[:, :, C * h:C * (h + 1)], in0=xv[:, :, C * h:C * (h + 1)],
                            in1=sv[:, :, HC + h:HC + h + 1].to_broadcast([128, NT, C]),
                            op=OP.mult)
                    for k in range(NT):
                        nc.tensor.matmul(nd[:], lhsT=a_sb[:, 128 * k:128 * (k + 1)],
                                         rhs=stg[:, SEC * k:SEC * (k + 1)],
                                         start=(k == 0), stop=(k == NT - 1))

                    # ---------------- per-block epilogue ----------------
                    if layer == 1:
                        cntc = sb.tile([BLKN, 1], F32, tag='cntc')
                        nc.vector.tensor_scalar(out=cntc[:], in0=nd[:BLKN, HC + H + ED:SEC],
                                                scalar1=1.0, scalar2=None, op0=OP.max)
                        rec = sb.tile([BLKN, 1], F32, tag='rec')
                        nc.vector.reciprocal(out=rec[:], in_=cntc[:])
                        nc.vector.tensor_scalar(out=mean_r[:, ED * b:ED * (b + 1)],
                                                in0=nd[:BLKN, HC + H:HC + H + ED],
                                                scalar1=rec[:], scalar2=None, op0=OP.mult)

                    mt_ps = spp.tile([ED, BLKN], F32, space='PSUM', tag='small')
                    nc.tensor.matmul(mt_ps[:], lhsT=mean_r[:, ED * b:ED * (b + 1)],
                                     rhs=CB['ident'][:BLKN, :BLKN],
                                     start=True, stop=True, is_transpose=True)
                    mt_sb = sb.tile([ED, BLKN], F32, tag='mt_sb')
                    nc.vector.tensor_copy(out=mt_sb[:], in_=mt_ps[:])
                    xls = sb.tile([BLKN, HC], F32, tag='xls')
                    nc.sync.dma_start(out=xls[:], in_=xl_own_src[BLKN * b:BLKN * (b + 1), :])
                    ms_ps = spp.tile([HC, BLKN], F32, space='PSUM', tag='small')
                    nc.tensor.matmul(ms_ps[:], lhsT=we_sb[:],
                                     rhs=mt_sb[:],
                                     start=True, stop=False)
                    nc.tensor.matmul(ms_ps[:], lhsT=xr[:, HC * b:HC * (b + 1)],
                                     rhs=CB['ident'][:BLKN, :BLKN], start=False, stop=False,
                                     is_transpose=True)
                    nc.tensor.matmul(ms_ps[:], lhsT=xls[:], rhs=CB['ident'][:BLKN, :BLKN],
                                     start=False, stop=True, is_transpose=True)
                    rps = sb.tile([HC, BLKN], F32, tag='rps')
                    rns = sb.tile([HC, BLKN], F32, tag='rns')
                    nc.scalar.activation(out=rps[:], in_=ms_ps[:], func=AF.Relu)
                    nc.scalar.activation(out=rns[:], in_=ms_ps[:], func=AF.Relu, scale=-1.0)
                    ws_ps = spp.tile([BLKN, H], F32, space='PSUM', tag='small')
                    nc.tensor.matmul(ws_ps[:], lhsT=rps[:], rhs=sgn_sb[:], start=True, stop=False)
                    nc.tensor.matmul(ws_ps[:], lhsT=rns[:], rhs=sgnn_sb[:], start=False, stop=True)
                    ws = sb.tile([BLKN, H], F32, tag='wsx')
                    nc.scalar.activation(out=ws[:], in_=ws_ps[:], func=AF.Exp)

                    den = sb.tile([BLKN, H], F32, tag='den')
                    nc.vector.tensor_tensor(out=den[:], in0=nd[:BLKN, HC:HC + H], in1=ws[:],
                                            op=OP.add)
                    rden = sb.tile([BLKN, H], F32, tag='rden')
                    nc.vector.reciprocal(out=rden[:], in_=den[:])
                    wxls = sb.tile([BLKN, HC], F32, tag='wxls')
                    nc.vector.tensor_tensor(
                        out=wxls[:], in0=xls[:],
                        in1=ws[:].rearrange('p (h u) -> p h u', u=1).to_broadcast([BLKN, H, C]),
                        op=OP.mult)
                    numt = sb.tile([BLKN, HC], F32, tag='numt')
                    nc.vector.tensor_tensor(out=numt[:], in0=nd[:BLKN, 0:HC], in1=wxls[:],
                                            op=OP.add)
                    onrm = sb.tile([BLKN, HC], F32, tag='onrm')
                    nc.vector.tensor_tensor(
                        out=onrm[:], in0=numt[:],
                        in1=rden[:].rearrange('p (h u) -> p h u', u=1).to_broadcast([BLKN, H, C]),
                        op=OP.mult)
                    tA = sb.tile([BLKN, C], F32, tag='tA')
                    nc.vector.tensor_tensor(out=tA[:], in0=onrm[:, 0:C], in1=ia_sb[:BLKN, 0:C],
                                            op=OP.mult)
                    tB = sb.tile([BLKN, C], F32, tag='tB')
                    nc.vector.tensor_tensor(out=tB[:], in0=onrm[:, C:HC], in1=ia_sb[:BLKN, C:HC],
                                            op=OP.mult)
                    tC = sb.tile([BLKN, C], F32, tag='tC')
                    nc.vector.tensor_tensor(out=tC[:], in0=tA[:], in1=tB[:], op=OP.add)
                    hbl = sb.tile([BLKN, C], F32, tag='hbl')
                    nc.vector.tensor_tensor(out=hbl[:], in0=tC[:], in1=bo_sb[:BLKN, :], op=OP.add)

                    if layer == 1:
                        nc.vector.tensor_copy(out=h_res[:, C * b:C * (b + 1)], in_=hbl[:])
                    else:
                        rl = sb.tile([BLKN, C], F32, tag='rl')
                        nc.scalar.activation(out=rl[:], in_=hbl[:], func=AF.Relu)
                        mn = sb.tile([BLKN, C], F32, tag='mn')
                        nc.vector.tensor_scalar(out=mn[:], in0=hbl[:], scalar1=0.0, scalar2=None,
                                                op0=OP.min)
                        pw = sb.tile([BLKN, C], F32, tag='pw')
                        nc.vector.tensor_tensor(out=pw[:], in0=mn[:], in1=CB['prelub'][:BLKN, :],
                                                op=OP.mult)
                        ybl = sb.tile([BLKN, C], F32, tag='ybl')
                        nc.vector.tensor_tensor(out=ybl[:], in0=rl[:], in1=pw[:], op=OP.add)
                        nc.sync.dma_start(out=y_d[BLKN * b:BLKN * (b + 1), :], in_=ybl[:])

        edge_pass(1)

        # ---------------- layer-2 projections + AllGather ----------------
        with (tc.tile_pool(name='pj2', bufs=3) as pj,
              tc.tile_pool(name='pjp2', bufs=2, space='PSUM') as pjp):
            for b in range(NBLK):
                ht_ps = pjp.tile([C, BLKN], F32, space='PSUM', tag='ht2')
                nc.tensor.matmul(ht_ps[:], lhsT=h_res[:, C * b:C * (b + 1)],
                                 rhs=CB['ident'][:BLKN, :BLKN], start=True, stop=True,
                                 is_transpose=True)
                hT_sl = pj.tile([C, BLKN], F32, tag='hT_sl')
                nc.vector.tensor_copy(out=hT_sl[:], in_=ht_ps[:])
                ps_xl = pjp.tile([BLKN, HC], F32, space='PSUM', tag='ps2_xl')
                ps_xr = pjp.tile([BLKN, HC], F32, space='PSUM', tag='ps2_xr')
                nc.tensor.matmul(ps_xl[:], lhsT=hT_sl[:], rhs=CB['wl2'][:], start=True, stop=True)
                nc.tensor.matmul(ps_xr[:], lhsT=hT_sl[:], rhs=CB['wr2'][:], start=True, stop=True)
                xl_sb = pj.tile([BLKN, HC], F32, tag='xl2_sb')
                nc.vector.tensor_tensor(out=xl_sb[:], in0=ps_xl[:], in1=CB['bl2b'][:BLKN, :],
                                        op=OP.add)
                nc.sync.dma_start(out=xl_bounce[2][BLKN * b:BLKN * (b + 1), :], in_=xl_sb[:])
                nc.vector.tensor_tensor(out=xr_res[2][:, HC * b:HC * (b + 1)], in0=ps_xr[:],
                                        in1=CB['br2b'][:BLKN, :], op=OP.add)

        nc.gpsimd.collective_compute(
            'AllGather', OP.bypass, replica_groups=[list(range(NC))],
            ins=[xl_bounce[2][:]], outs=[xl_table[2][:]])

        edge_pass(2)

        dp.release()
        rp.release()
        cp.release()

    nc.compile()
    return nc


_CACHE = {}

def kernel(**inputs):
    cfg = FULL
    key = 'full'
    if key not in _CACHE:
        _CACHE[key] = build_program(cfg)
    nc = _CACHE[key]
    in_maps = stage_inputs(cfg, inputs)
    import concourse.bass_utils as bass_utils
    res = bass_utils.run_bass_kernel_spmd(nc, in_maps, core_ids=list(range(cfg.NCORES)))
    return np.concatenate([res.results[c]['y'] for c in range(cfg.NCORES)], axis=0)
